# revision 27
# baseline (speedup 1.0000x reference)
"""ChebConv (K=3, two layers) GNN kernel for 8 Trainium2 NeuronCores.

Strategy (graph/data parallel, per sharding hint):
  - Nodes are partitioned into 8 contiguous ranges (12500 per core); each core
    owns the scatter-sum output for its dst range.
  - Edges are bucketed by (dst block of 128, src split of 25000) on the host;
    per bucket, edges are packed into chunks of 128 (padded).
  - One propagate (h -> segment_sum(norm * h[src], dst)) per dst block:
      * SWDGE dma_gather of the (pre-scaled, bf16) source rows, one
        instruction per (block, split) since indices are int16
      * selection matrices S[e, d] = (slot[e] == d) built with a single
        broadcast is_equal on the vector engine (all-bf16)
      * TensorE matmuls S^T @ G accumulate the per-dst-slot sums in PSUM
  - All gather indices and slot metadata are SBUF-resident (loaded once,
    reused by all 4 propagates).
  - The symmetric normalization -dinv[src]*dinv[dst] is folded in by storing
    gather tables pre-scaled by dinv (u = dinv * h) and scaling the PSUM
    result by -dinv[dst] on evacuation (scalar engine, per-partition scale).
  - The Chebyshev recurrence subtraction (Tx2 = 2*prop(Tx1) - Tx0) is folded
    into the dense weights: out = Tx0 @ (W0 - W2) + Tx1 @ W1 + p @ W2 with
    p = 2*prop(Tx1), so Tx0 is never reloaded in the second epilogue.
  - After each propagate whose result other cores need, an AllGather
    replicates the scaled table (bf16) to every core.
"""

import math
import os

import numpy as np

P = 128


def _ceil_div(a, b):
    return (a + b - 1) // b


def build_program(cfg, x, edge_index, W1, b1, W2, b2):
    import concourse.bacc as bacc
    import concourse.tile as tile
    from concourse import bass, mybir
    from concourse.masks import make_identity

    f32 = mybir.dt.float32
    bf16 = mybir.dt.bfloat16
    i32 = mybir.dt.int32
    i16 = mybir.dt.int16
    bf16_np = mybir.dt.np(bf16)
    AF = mybir.ActivationFunctionType
    OP = mybir.AluOpType

    N = cfg["N"]
    E = cfg["E"]
    IN = cfg["IN"]
    HID = cfg["HID"]
    OUT = cfg["OUT"]
    ncores = cfg["ncores"]
    n_loc = N // ncores
    assert n_loc * ncores == N
    nb = _ceil_div(n_loc, P)
    nbP = nb * P
    TF = 128  # gather-table row width (256B rows; SWDGE is desc-rate bound)

    x = np.asarray(x, dtype=np.float32)
    src = np.asarray(edge_index[0]).astype(np.int64)
    dst = np.asarray(edge_index[1]).astype(np.int64)
    W1 = np.asarray(W1, dtype=np.float32)
    b1 = np.asarray(b1, dtype=np.float32)
    W2 = np.asarray(W2, dtype=np.float32)
    b2 = np.asarray(b2, dtype=np.float32)

    # ---- host-side graph preprocessing (sharding prep) ----
    deg = np.bincount(src, minlength=N).astype(np.float32)
    dinv = np.where(deg > 0, 1.0 / np.sqrt(np.maximum(deg, 1.0)), 0.0).astype(
        np.float32
    )

    # Gather tables are indexed with int16 (dma_gather), so split node space
    # into NSPLIT sub-tables. Splits are defined by OWNER-LOCAL row ranges
    # (pieces of 25 dst blocks), so each split's table slice is produced by
    # one piece-wise AllGather and the next propagate's split-q gathers only
    # wait on piece q.
    NSPLIT = 4
    rows_split = _ceil_div(N, NSPLIT)
    assert rows_split <= 32000

    qsplit = src // rows_split
    lidx16 = (src - qsplit * rows_split).astype(np.int16)

    core_all = dst // n_loc
    loc = dst - core_all * n_loc
    blk_all = loc // P
    slot_all = (loc - blk_all * P).astype(np.float32)

    key = (core_all * nb + blk_all) * NSPLIT + qsplit
    order = np.argsort(key, kind="stable")
    key = key[order]
    s_lidx = lidx16[order]
    s_slot = slot_all[order]
    core_of = core_all[order]
    blk = blk_all[order]
    qs = qsplit[order]

    counts = np.bincount(key, minlength=ncores * nb * NSPLIT).reshape(
        ncores, nb, NSPLIT
    )
    # chunks per (block, split): shared across cores (SPMD program)
    CQ = _ceil_div(counts.max(axis=0), P)  # [nb, NSPLIT]
    cqcum = np.zeros((nb, NSPLIT + 1), dtype=np.int64)
    np.cumsum(CQ, axis=1, out=cqcum[:, 1:])
    Ctot = cqcum[:, NSPLIT]  # chunks per block
    Cmax = int(Ctot.max())
    ctoff = np.zeros(nb + 1, dtype=np.int64)  # chunk offset of block in flat S meta
    np.cumsum(Ctot, out=ctoff[1:])
    nchunks = int(ctoff[nb])

    starts = np.zeros(ncores * nb * NSPLIT, dtype=np.int64)
    cnt_flat = counts.reshape(-1)
    np.cumsum(cnt_flat[:-1], out=starts[1:])
    j = np.arange(E, dtype=np.int64) - starts[key]
    chunk_l = cqcum[blk, qs] + j // P  # chunk id within block
    lane = j % P

    # slot metadata, flat over (block, chunk): bf16, sentinel 300 => no slot
    slotv = np.full((ncores, P, nchunks), 300.0, dtype=np.float32)
    slotv[core_of, lane, ctoff[blk] + chunk_l] = s_slot
    slots_bf = slotv.astype(bf16_np)

    # int16 gather indices in 16-partition wrap, replicated to 128 partitions,
    # flat over (block, split) segments of 8*CQ[b,q] columns each.
    # Padding positions hold -1 (skipped by SWDGE when num_idxs_reg gives the
    # per-core valid count).
    icoff = np.zeros((nb, NSPLIT + 1), dtype=np.int64)
    icoff[:, 1:] = 8 * cqcum[:, 1:]
    base_icoff = np.zeros(nb + 1, dtype=np.int64)
    np.cumsum(8 * Ctot, out=base_icoff[1:])
    ticols = int(base_icoff[nb])
    gidx16 = np.zeros((ncores, 16, ticols), dtype=np.int16)
    L = (chunk_l - cqcum[blk, qs]) * P + j % P  # position within (b,q) gather
    col = base_icoff[blk] + icoff[blk, qs] + L // 16
    gidx16[core_of, L % 16, col] = s_lidx
    gidx = np.tile(gidx16, (1, 8, 1))  # [ncores, 128, ticols]

    # per-block scale columns: [nd, nd2, n2d, d] = [-dinv, -dinv^2, -2dinv, dinv]
    tmp = dinv.reshape(ncores, n_loc)
    pad = np.zeros((ncores, nbP - n_loc), dtype=np.float32)
    dv = np.concatenate([tmp, pad], axis=1).reshape(ncores, nb, P)  # [c, b, p]
    scales = np.zeros((ncores, P, nb, 4), dtype=np.float32)
    scales[:, :, :, 0] = -dv.transpose(0, 2, 1)
    scales[:, :, :, 1] = -(dv**2).transpose(0, 2, 1)
    scales[:, :, :, 2] = -2.0 * dv.transpose(0, 2, 1)
    scales[:, :, :, 3] = dv.transpose(0, 2, 1)
    scales = scales.reshape(ncores, P, nb * 4)

    xpad = np.concatenate(
        [x.reshape(ncores, n_loc, IN), np.zeros((ncores, nbP - n_loc, IN), np.float32)],
        axis=1,
    )
    xT = np.ascontiguousarray(xpad.transpose(0, 2, 1))  # [ncores, IN, nbP]
    u0 = (dinv[:, None] * x).astype(bf16_np)  # pre-scaled gather table, bf16

    # fold the Chebyshev subtraction into the k=0 weights
    W1f = W1.copy()
    W1f[0] = W1[0] - W1[2]
    W2f = W2.copy()
    W2f[0] = W2[0] - W2[2]

    # ---- build the SPMD program ----
    from concourse import library_config

    nc = bacc.Bacc(
        "TRN2",
        target_bir_lowering=False,
        debug=False,
        num_devices=ncores,
        num_swdge_queues=4,
    )

    u0_d = nc.dram_tensor("u0", [N, IN], bf16, kind="ExternalInput").ap()
    xT_d = nc.dram_tensor("xT", [IN, nbP], f32, kind="ExternalInput").ap()
    gidx_d = nc.dram_tensor("gidx", [P, ticols], i16, kind="ExternalInput").ap()
    slots_d = nc.dram_tensor("slots", [P, nchunks], bf16, kind="ExternalInput").ap()
    scales_d = nc.dram_tensor("scales", [P, nb * 4], f32, kind="ExternalInput").ap()
    w1_d = nc.dram_tensor("w1", [3, IN, HID], f32, kind="ExternalInput").ap()
    b1_d = nc.dram_tensor("b1", [HID], f32, kind="ExternalInput").ap()
    w2_d = nc.dram_tensor("w2", [3, HID, OUT], f32, kind="ExternalInput").ap()
    b2_d = nc.dram_tensor("b2", [OUT], f32, kind="ExternalInput").ap()
    out_d = nc.dram_tensor("out", [n_loc, OUT], f32, kind="ExternalOutput").ap()

    groups = [list(range(ncores))]

    from contextlib import ExitStack

    with ExitStack() as ctx:
        tc = ctx.enter_context(tile.TileContext(nc))

        dram = ctx.enter_context(tc.tile_pool(name="dram", bufs=1, space="DRAM"))
        u1_full = nc.dram_tensor("u1_full", [N, IN], bf16, addr_space="Shared")
        uh_full = nc.dram_tensor("uh_full", [N, TF], bf16, addr_space="Shared")
        ut1_full = nc.dram_tensor("ut1_full", [N, TF], bf16, addr_space="Shared")
        u1_loc = dram.tile([n_loc, IN], bf16, tag="u1_loc")
        uh_loc = dram.tile([n_loc, TF], bf16, tag="uh_loc")
        ut1_loc = dram.tile([n_loc, TF], bf16, tag="ut1_loc")
        tx1_loc = dram.tile([nbP, IN], f32, tag="tx1_loc")
        h_loc = dram.tile([nbP, HID], f32, tag="h_loc")
        th1_loc = dram.tile([nbP, HID], f32, tag="th1_loc")

        const = ctx.enter_context(tc.tile_pool(name="const", bufs=1))
        io = ctx.enter_context(tc.tile_pool(name="io", bufs=4))
        gp = ctx.enter_context(tc.tile_pool(name="gp", bufs=cfg.get("gbufs", 8)))
        sp = ctx.enter_context(tc.tile_pool(name="sp", bufs=cfg.get("sbufs", 4)))
        ev = ctx.enter_context(tc.tile_pool(name="ev", bufs=6))
        pps = ctx.enter_context(
            tc.tile_pool(name="pps", bufs=cfg.get("pbufs", 4), space="PSUM")
        )
        tps = ctx.enter_context(tc.tile_pool(name="tps", bufs=2, space="PSUM"))
        dps = ctx.enter_context(tc.tile_pool(name="dps", bufs=2, space="PSUM"))

        ident = const.tile([P, P], f32, tag="ident")
        make_identity(nc, ident[:])
        iota_i = const.tile([P, P], i32, tag="iota_i")
        nc.gpsimd.iota(iota_i[:], pattern=[[1, P]], base=0, channel_multiplier=0)
        iota_b = const.tile([P, P], bf16, tag="iota_b")
        nc.vector.tensor_copy(iota_b[:], iota_i[:])
        nc.gpsimd.load_library(library_config.mlp)

        # resident index/metadata tiles
        gix = const.tile([P, ticols], i16, tag="gix")
        nc.sync.dma_start(gix[:], gidx_d[:])
        slots_t = const.tile([P, nchunks], bf16, tag="slots")
        nc.sync.dma_start(slots_t[:], slots_d[:])
        scl = const.tile([P, nb * 4], f32, tag="scl")
        nc.sync.dma_start(scl[:], scales_d[:])

        w1_t = []
        for k in range(3):
            t = const.tile([IN, HID], f32, tag=f"w1_{k}")
            nc.sync.dma_start(t[:], w1_d[k])
            w1_t.append(t)
        w2_t = []
        for k in range(3):
            t = const.tile([HID, OUT], f32, tag=f"w2_{k}")
            nc.sync.dma_start(t[:], w2_d[k])
            w2_t.append(t)
        ones1 = const.tile([1, P], f32, tag="ones1")
        nc.vector.memset(ones1[:], 1.0)
        b1_t = const.tile([1, HID], f32, tag="b1_t")
        nc.sync.dma_start(b1_t[:], b1_d[None, :])
        b2_t = const.tile([1, OUT], f32, tag="b2_t")
        nc.sync.dma_start(b2_t[:], b2_d[None, :])


        def propagate(table_ap, F, epilogue):
            # table_ap: [N, TF] bf16; F = feature cols actually used (<= TF)
            PC = 3  # chunks per gather piece: 25 descs -> 2 fit per 64-desc ring
            for b in range(nb):
                CT = int(Ctot[b])
                G = gp.tile([P, Cmax * TF], bf16, tag="G")
                work = []
                for q in range(NSPLIT):
                    cgq = int(CQ[b, q])
                    p0 = 0
                    while p0 < cgq:
                        work.append((q, p0, min(PC, cgq - p0)))
                        p0 += PC
                work.sort(key=lambda t: (t[1], t[0]))
                for q, p0, pn in work:
                    c0 = int(cqcum[b, q]) + p0
                    n_q = pn * P
                    r0 = q * rows_split
                    r1 = min(r0 + rows_split, N)
                    ic0 = int(base_icoff[b] + icoff[b, q]) + 8 * p0
                    nc.gpsimd.dma_gather(
                        G[:, c0 * TF : (c0 + pn) * TF].rearrange(
                            "p (c f) -> p c f", f=TF
                        ),
                        table_ap[r0:r1],
                        gix[:, ic0 : ic0 + 8 * pn],
                        n_q,
                        n_q,
                        TF,
                        queue_num=q,
                        single_packet=True,
                    )
                S = sp.tile([P, Cmax * P], bf16, tag="S")
                so = int(ctoff[b])
                nc.vector.tensor_tensor(
                    out=S[:, : CT * P].rearrange("p (c q) -> p c q", q=P),
                    in0=slots_t[:, so : so + CT].to_broadcast([P, CT, P]),
                    in1=iota_b[:, None, :].to_broadcast([P, CT, P]),
                    op=OP.is_equal,
                )
                ps = pps.tile([P, F], f32, tag="prop_ps")
                for kk in range(CT):
                    nc.tensor.matmul(
                        out=ps[:],
                        lhsT=S[:, kk * P : (kk + 1) * P],
                        rhs=G[:, kk * TF : kk * TF + F],
                        start=(kk == 0),
                        stop=(kk == CT - 1),
                    )
                epilogue(b, ps)
            # scale column APs for block b: scl[:, 4b+j : 4b+j+1]

        def rows_of(b):
            return min(P, n_loc - b * P)


        def sc(b, j):
            return scl[:, 4 * b + j : 4 * b + j + 1]

        # ---- layer 1, propagate #1: Tx1 = -Ds A Ds x ----
        def epi1(b, ps):
            rows = rows_of(b)
            tx1 = ev.tile([P, IN], f32, tag="tx1")
            nc.scalar.activation(tx1[:], ps[:], AF.Copy, scale=sc(b, 0))
            u1 = ev.tile([P, IN], bf16, tag="u1")
            nc.scalar.activation(u1[:], ps[:], AF.Copy, scale=sc(b, 1))
            nc.sync.dma_start(tx1_loc[:][b * P : b * P + P], tx1[:])
            nc.sync.dma_start(u1_loc[:][b * P : b * P + rows], u1[:rows])

        propagate(u0_d, IN, epi1)
        nc.gpsimd.collective_compute(
            "AllGather", OP.bypass, replica_groups=groups,
            ins=[u1_loc.opt()], outs=[u1_full.ap()],
        )

        # ---- layer 1, propagate #2 + dense layer 1 ----
        def epi2(b, ps):
            rows = rows_of(b)
            p2 = ev.tile([P, IN], f32, tag="p2")
            nc.scalar.activation(p2[:], ps[:], AF.Copy, scale=sc(b, 2))
            xT_t = io.tile([IN, P], f32, tag="xT_t")
            nc.sync.dma_start(xT_t[:], xT_d[:, b * P : (b + 1) * P])
            tx1 = io.tile([P, IN], f32, tag="tx1b")
            nc.sync.dma_start(tx1[:], tx1_loc[:][b * P : (b + 1) * P])
            outps = dps.tile([P, HID], f32, tag="dps")
            nc.tensor.matmul(
                out=outps[:], lhsT=xT_t[:], rhs=w1_t[0][:],
                start=True, stop=False, skip_group_check=True,
            )
            for k, t in [(1, tx1), (2, p2)]:
                tp = tps.tile([IN, P], f32, tag="trp")
                nc.tensor.transpose(tp[:], t[:], ident[:])
                tsb = ev.tile([IN, P], f32, tag="trs")
                nc.scalar.activation(tsb[:], tp[:], AF.Copy)
                nc.tensor.matmul(
                    out=outps[:], lhsT=tsb[:], rhs=w1_t[k][:],
                    start=False, stop=False, skip_group_check=True,
                )
            nc.tensor.matmul(
                out=outps[:], lhsT=ones1[:1, :], rhs=b1_t[:1, :],
                start=False, stop=True, skip_group_check=True,
            )
            h_t = ev.tile([P, HID], f32, tag="h_t")
            nc.scalar.activation(h_t[:], outps[:], AF.Relu)
            uh = ev.tile([P, TF], bf16, tag="uh")
            nc.vector.memset(uh[:, HID:TF], 0.0)
            nc.scalar.activation(uh[:, 0:HID], h_t[:], AF.Copy, scale=sc(b, 3))
            nc.sync.dma_start(h_loc[:][b * P : b * P + P], h_t[:])
            nc.sync.dma_start(uh_loc[:][b * P : b * P + rows], uh[:rows])

        propagate(u1_full.ap(), IN, epi2)
        nc.gpsimd.collective_compute(
            "AllGather", OP.bypass, replica_groups=groups,
            ins=[uh_loc.opt()], outs=[uh_full.ap()],
        )

        # ---- layer 2, propagate #1: Th1 ----
        def epi3(b, ps):
            rows = rows_of(b)
            th1 = ev.tile([P, HID], f32, tag="th1")
            nc.scalar.activation(th1[:], ps[:], AF.Copy, scale=sc(b, 0))
            ut1 = ev.tile([P, TF], bf16, tag="ut1")
            nc.vector.memset(ut1[:, HID:TF], 0.0)
            nc.scalar.activation(ut1[:, 0:HID], ps[:], AF.Copy, scale=sc(b, 1))
            nc.sync.dma_start(th1_loc[:][b * P : b * P + P], th1[:])
            nc.sync.dma_start(ut1_loc[:][b * P : b * P + rows], ut1[:rows])

        propagate(uh_full.ap(), HID, epi3)
        nc.gpsimd.collective_compute(
            "AllGather", OP.bypass, replica_groups=groups,
            ins=[ut1_loc.opt()], outs=[ut1_full.ap()],
        )

        # ---- layer 2, propagate #2 + dense layer 2 + output ----
        def epi4(b, ps):
            rows = rows_of(b)
            p4 = ev.tile([P, HID], f32, tag="p4")
            nc.scalar.activation(p4[:], ps[:], AF.Copy, scale=sc(b, 2))
            h_t = io.tile([P, HID], f32, tag="h_t2")
            nc.sync.dma_start(h_t[:], h_loc[:][b * P : (b + 1) * P])
            th1 = io.tile([P, HID], f32, tag="th1b")
            nc.sync.dma_start(th1[:], th1_loc[:][b * P : (b + 1) * P])
            outps = dps.tile([P, OUT], f32, tag="dps")
            for k, t in [(0, h_t), (1, th1), (2, p4)]:
                tp = tps.tile([HID, P], f32, tag="trp")
                nc.tensor.transpose(tp[:], t[:], ident[:])
                tsb = ev.tile([HID, P], f32, tag="trs2")
                nc.scalar.activation(tsb[:], tp[:], AF.Copy)
                nc.tensor.matmul(
                    out=outps[:], lhsT=tsb[:], rhs=w2_t[k][:],
                    start=(k == 0), stop=False, skip_group_check=True,
                )
            nc.tensor.matmul(
                out=outps[:], lhsT=ones1[:1, :], rhs=b2_t[:1, :],
                start=False, stop=True, skip_group_check=True,
            )
            o_t = ev.tile([P, OUT], f32, tag="o_t")
            nc.scalar.activation(o_t[:], outps[:], AF.Copy)
            nc.sync.dma_start(out_d[b * P : b * P + rows], o_t[:rows])

        propagate(ut1_full.ap(), HID, epi4)

    nc.compile()

    in_map = lambda c: {
        "u0": u0,
        "xT": np.ascontiguousarray(xT[c]),
        "gidx": np.ascontiguousarray(gidx[c]),
        "slots": np.ascontiguousarray(slots_bf[c]),
        "scales": np.ascontiguousarray(scales[c]),
        "w1": W1f,
        "b1": b1,
        "w2": W2f,
        "b2": b2,
    }
    in_maps = [in_map(c) for c in range(ncores)]
    return nc, in_maps


def build_and_run(cfg, x, edge_index, W1, b1, W2, b2, trace=False):
    from concourse.bass_utils import run_bass_kernel_spmd

    ncores = cfg["ncores"]
    nc, in_maps = build_program(cfg, x, edge_index, W1, b1, W2, b2)
    res = run_bass_kernel_spmd(nc, in_maps, list(range(ncores)), trace=trace)
    out = np.concatenate([res.results[c]["out"] for c in range(ncores)], axis=0)
    return out, res


def kernel(x, edge_index, W1, b1, W2, b2):
    cfg = dict(N=100000, E=1600000, IN=128, HID=64, OUT=40, ncores=8)
    trace = os.environ.get("CHEB_TRACE", "0") == "1"
    out, res = build_and_run(cfg, x, edge_index, W1, b1, W2, b2, trace=trace)
    if trace and res.exec_time_ns is not None:
        print(f"HW exec time: {res.exec_time_ns} ns")
    return out



# revision 31
# speedup vs baseline: 1.2214x; 1.2214x over previous
"""ChebConv (K=3, two layers) GNN kernel for 8 Trainium2 NeuronCores.

Strategy (graph/data parallel, per sharding hint):
  - Nodes are partitioned into 8 contiguous ranges (12500 per core); each core
    owns the scatter-sum output for its dst range.
  - Edges are bucketed by (dst block of 128, src split of 25000) on the host;
    per bucket, edges are packed into chunks of 128 (padded).
  - One propagate (h -> segment_sum(norm * h[src], dst)) per dst block:
      * SWDGE dma_gather of the (pre-scaled, bf16) source rows, one
        instruction per (block, split) since indices are int16
      * selection matrices S[e, d] = (slot[e] == d) built with a single
        broadcast is_equal on the vector engine (all-bf16)
      * TensorE matmuls S^T @ G accumulate the per-dst-slot sums in PSUM
  - All gather indices and slot metadata are SBUF-resident (loaded once,
    reused by all 4 propagates).
  - The symmetric normalization -dinv[src]*dinv[dst] is folded in by storing
    gather tables pre-scaled by dinv (u = dinv * h) and scaling the PSUM
    result by -dinv[dst] on evacuation (scalar engine, per-partition scale).
  - The Chebyshev recurrence subtraction (Tx2 = 2*prop(Tx1) - Tx0) is folded
    into the dense weights: out = Tx0 @ (W0 - W2) + Tx1 @ W1 + p @ W2 with
    p = 2*prop(Tx1), so Tx0 is never reloaded in the second epilogue.
  - After each propagate whose result other cores need, an AllGather
    replicates the scaled table (bf16) to every core.
"""

import math
import os

import numpy as np

P = 128


def _ceil_div(a, b):
    return (a + b - 1) // b


def build_program(cfg, x, edge_index, W1, b1, W2, b2):
    import concourse.bacc as bacc
    import concourse.tile as tile
    from concourse import bass, mybir
    from concourse.masks import make_identity

    f32 = mybir.dt.float32
    bf16 = mybir.dt.bfloat16
    i32 = mybir.dt.int32
    i16 = mybir.dt.int16
    bf16_np = mybir.dt.np(bf16)
    AF = mybir.ActivationFunctionType
    OP = mybir.AluOpType

    N = cfg["N"]
    E = cfg["E"]
    IN = cfg["IN"]
    HID = cfg["HID"]
    OUT = cfg["OUT"]
    ncores = cfg["ncores"]
    n_loc = N // ncores
    assert n_loc * ncores == N
    nb = _ceil_div(n_loc, P)
    nbP = nb * P
    TF = 128  # gather-table row width (256B rows; SWDGE is desc-rate bound)

    x = np.asarray(x, dtype=np.float32)
    src = np.asarray(edge_index[0]).astype(np.int64)
    dst = np.asarray(edge_index[1]).astype(np.int64)
    W1 = np.asarray(W1, dtype=np.float32)
    b1 = np.asarray(b1, dtype=np.float32)
    W2 = np.asarray(W2, dtype=np.float32)
    b2 = np.asarray(b2, dtype=np.float32)

    # ---- host-side graph preprocessing (sharding prep) ----
    deg = np.bincount(src, minlength=N).astype(np.float32)
    dinv = np.where(deg > 0, 1.0 / np.sqrt(np.maximum(deg, 1.0)), 0.0).astype(
        np.float32
    )

    # Gather tables are indexed with int16 (dma_gather), so split node space
    # into NSPLIT sub-tables. Splits are defined by OWNER-LOCAL row ranges
    # (pieces of 25 dst blocks), so each split's table slice is produced by
    # one piece-wise AllGather and the next propagate's split-q gathers only
    # wait on piece q.
    NSPLIT = 4
    rows_split = _ceil_div(N, NSPLIT)
    assert rows_split <= 32000

    qsplit = src // rows_split
    lidx16 = (src - qsplit * rows_split).astype(np.int16)

    core_all = dst // n_loc
    loc = dst - core_all * n_loc
    blk_all = loc // P
    slot_all = (loc - blk_all * P).astype(np.float32)

    key = (core_all * nb + blk_all) * NSPLIT + qsplit
    order = np.argsort(key, kind="stable")
    key = key[order]
    s_lidx = lidx16[order]
    s_slot = slot_all[order]
    core_of = core_all[order]
    blk = blk_all[order]
    qs = qsplit[order]

    counts = np.bincount(key, minlength=ncores * nb * NSPLIT).reshape(
        ncores, nb, NSPLIT
    )
    # chunks per (block, split): shared across cores (SPMD program)
    CQ = _ceil_div(counts.max(axis=0), P)  # [nb, NSPLIT]
    cqcum = np.zeros((nb, NSPLIT + 1), dtype=np.int64)
    np.cumsum(CQ, axis=1, out=cqcum[:, 1:])
    Ctot = cqcum[:, NSPLIT]  # chunks per block
    Cmax = int(Ctot.max())
    ctoff = np.zeros(nb + 1, dtype=np.int64)  # chunk offset of block in flat S meta
    np.cumsum(Ctot, out=ctoff[1:])
    nchunks = int(ctoff[nb])

    starts = np.zeros(ncores * nb * NSPLIT, dtype=np.int64)
    cnt_flat = counts.reshape(-1)
    np.cumsum(cnt_flat[:-1], out=starts[1:])
    j = np.arange(E, dtype=np.int64) - starts[key]
    chunk_l = cqcum[blk, qs] + j // P  # chunk id within block
    lane = j % P

    # slot metadata, flat over (block, chunk): bf16, sentinel 300 => no slot
    slotv = np.full((ncores, P, nchunks), 300.0, dtype=np.float32)
    slotv[core_of, lane, ctoff[blk] + chunk_l] = s_slot
    slots_bf = slotv.astype(bf16_np)

    # int16 gather indices in 16-partition wrap, replicated to 128 partitions,
    # flat over (block, split) segments of 8*CQ[b,q] columns each.
    # Padding positions hold -1 (skipped by SWDGE when num_idxs_reg gives the
    # per-core valid count).
    icoff = np.zeros((nb, NSPLIT + 1), dtype=np.int64)
    icoff[:, 1:] = 8 * cqcum[:, 1:]
    base_icoff = np.zeros(nb + 1, dtype=np.int64)
    np.cumsum(8 * Ctot, out=base_icoff[1:])
    ticols = int(base_icoff[nb])
    gidx16 = np.zeros((ncores, 16, ticols), dtype=np.int16)
    L = (chunk_l - cqcum[blk, qs]) * P + j % P  # position within (b,q) gather
    col = base_icoff[blk] + icoff[blk, qs] + L // 16
    gidx16[core_of, L % 16, col] = s_lidx
    gidx = np.tile(gidx16, (1, 8, 1))  # [ncores, 128, ticols]

    # per-block scale columns: [nd, nd2, n2d, d] = [-dinv, -dinv^2, -2dinv, dinv]
    tmp = dinv.reshape(ncores, n_loc)
    pad = np.zeros((ncores, nbP - n_loc), dtype=np.float32)
    dv = np.concatenate([tmp, pad], axis=1).reshape(ncores, nb, P)  # [c, b, p]
    scales = np.zeros((ncores, P, nb, 4), dtype=np.float32)
    scales[:, :, :, 0] = -dv.transpose(0, 2, 1)
    scales[:, :, :, 1] = -(dv**2).transpose(0, 2, 1)
    scales[:, :, :, 2] = -2.0 * dv.transpose(0, 2, 1)
    scales[:, :, :, 3] = dv.transpose(0, 2, 1)
    scales = scales.reshape(ncores, P, nb * 4)

    xpad = np.concatenate(
        [x.reshape(ncores, n_loc, IN), np.zeros((ncores, nbP - n_loc, IN), np.float32)],
        axis=1,
    )
    xT = np.ascontiguousarray(xpad.transpose(0, 2, 1))  # [ncores, IN, nbP]
    u0 = (dinv[:, None] * x).astype(bf16_np)  # pre-scaled gather table, bf16

    # fold the Chebyshev subtraction into the k=0 weights
    W1f = W1.copy()
    W1f[0] = W1[0] - W1[2]
    W2f = W2.copy()
    W2f[0] = W2[0] - W2[2]

    # ---- build the SPMD program ----
    from concourse import library_config

    nc = bacc.Bacc(
        "TRN2",
        target_bir_lowering=False,
        debug=False,
        num_devices=ncores,
        num_swdge_queues=4,
    )

    u0_d = nc.dram_tensor("u0", [N, IN], bf16, kind="ExternalInput").ap()
    xT_d = nc.dram_tensor("xT", [IN, nbP], f32, kind="ExternalInput").ap()
    gidx_d = nc.dram_tensor("gidx", [P, ticols], i16, kind="ExternalInput").ap()
    slots_d = nc.dram_tensor("slots", [P, nchunks], bf16, kind="ExternalInput").ap()
    scales_d = nc.dram_tensor("scales", [P, nb * 4], f32, kind="ExternalInput").ap()
    w1_d = nc.dram_tensor("w1", [3, IN, HID], f32, kind="ExternalInput").ap()
    b1_d = nc.dram_tensor("b1", [HID], f32, kind="ExternalInput").ap()
    w2_d = nc.dram_tensor("w2", [3, HID, OUT], f32, kind="ExternalInput").ap()
    b2_d = nc.dram_tensor("b2", [OUT], f32, kind="ExternalInput").ap()
    out_d = nc.dram_tensor("out", [n_loc, OUT], f32, kind="ExternalOutput").ap()

    groups = [list(range(ncores))]

    from contextlib import ExitStack

    with ExitStack() as ctx:
        tc = ctx.enter_context(tile.TileContext(nc))

        dram = ctx.enter_context(tc.tile_pool(name="dram", bufs=1, space="DRAM"))
        u1_full = nc.dram_tensor("u1_full", [N, IN], bf16, addr_space="Shared")
        uh_full = nc.dram_tensor("uh_full", [N, TF], bf16, addr_space="Shared")
        ut1_full = nc.dram_tensor("ut1_full", [N, TF], bf16, addr_space="Shared")
        u1_loc = dram.tile([n_loc, IN], bf16, tag="u1_loc")
        uh_loc = dram.tile([n_loc, TF], bf16, tag="uh_loc")
        ut1_loc = dram.tile([n_loc, TF], bf16, tag="ut1_loc")
        tx1_loc = dram.tile([nbP, IN], f32, tag="tx1_loc")
        h_loc = dram.tile([nbP, HID], f32, tag="h_loc")
        th1_loc = dram.tile([nbP, HID], f32, tag="th1_loc")

        const = ctx.enter_context(tc.tile_pool(name="const", bufs=1))
        io = ctx.enter_context(tc.tile_pool(name="io", bufs=4))
        gp = ctx.enter_context(tc.tile_pool(name="gp", bufs=cfg.get("gbufs", 8)))
        sp = ctx.enter_context(tc.tile_pool(name="sp", bufs=cfg.get("sbufs", 4)))
        ev = ctx.enter_context(tc.tile_pool(name="ev", bufs=6))
        pps = ctx.enter_context(
            tc.tile_pool(name="pps", bufs=cfg.get("pbufs", 4), space="PSUM")
        )
        tps = ctx.enter_context(tc.tile_pool(name="tps", bufs=2, space="PSUM"))
        dps = ctx.enter_context(tc.tile_pool(name="dps", bufs=2, space="PSUM"))

        ident = const.tile([P, P], f32, tag="ident")
        make_identity(nc, ident[:])
        iota_i = const.tile([P, P], i32, tag="iota_i")
        nc.gpsimd.iota(iota_i[:], pattern=[[1, P]], base=0, channel_multiplier=0)
        iota_b = const.tile([P, P], bf16, tag="iota_b")
        nc.vector.tensor_copy(iota_b[:], iota_i[:])
        nc.gpsimd.load_library(library_config.mlp)

        # resident index/metadata tiles
        gix = const.tile([P, ticols], i16, tag="gix")
        nc.sync.dma_start(gix[:], gidx_d[:])
        slots_t = const.tile([P, nchunks], bf16, tag="slots")
        nc.sync.dma_start(slots_t[:], slots_d[:])
        scl = const.tile([P, nb * 4], f32, tag="scl")
        nc.sync.dma_start(scl[:], scales_d[:])

        w1_t = []
        for k in range(3):
            t = const.tile([IN, HID], f32, tag=f"w1_{k}")
            nc.sync.dma_start(t[:], w1_d[k])
            w1_t.append(t)
        w2_t = []
        for k in range(3):
            t = const.tile([HID, OUT], f32, tag=f"w2_{k}")
            nc.sync.dma_start(t[:], w2_d[k])
            w2_t.append(t)
        ones1 = const.tile([1, P], f32, tag="ones1")
        nc.vector.memset(ones1[:], 1.0)
        b1_t = const.tile([1, HID], f32, tag="b1_t")
        nc.sync.dma_start(b1_t[:], b1_d[None, :])
        b2_t = const.tile([1, OUT], f32, tag="b2_t")
        nc.sync.dma_start(b2_t[:], b2_d[None, :])


        def propagate(table_ap, F, epilogue):
            # table_ap: [N, TF] bf16; F = feature cols actually used (<= TF)
            for b in range(nb):
                CT = int(Ctot[b])
                G = gp.tile([P, Cmax * TF], bf16, tag="G")
                for q in range(NSPLIT):
                    cgq = int(CQ[b, q])
                    if cgq == 0:
                        continue
                    c0 = int(cqcum[b, q])
                    n_q = cgq * P
                    r0 = q * rows_split
                    r1 = min(r0 + rows_split, N)
                    ic0 = int(base_icoff[b] + icoff[b, q])
                    nc.gpsimd.dma_gather(
                        G[:, c0 * TF : (c0 + cgq) * TF].rearrange(
                            "p (c f) -> p c f", f=TF
                        ),
                        table_ap[r0:r1],
                        gix[:, ic0 : ic0 + 8 * cgq],
                        n_q,
                        n_q,
                        TF,
                        queue_num=q,
                        single_packet=True,
                    )
                S = sp.tile([P, Cmax * P], bf16, tag="S")
                so = int(ctoff[b])
                nc.vector.tensor_tensor(
                    out=S[:, : CT * P].rearrange("p (c q) -> p c q", q=P),
                    in0=slots_t[:, so : so + CT].to_broadcast([P, CT, P]),
                    in1=iota_b[:, None, :].to_broadcast([P, CT, P]),
                    op=OP.is_equal,
                )
                ps = pps.tile([P, F], f32, tag="prop_ps")
                for kk in range(CT):
                    nc.tensor.matmul(
                        out=ps[:],
                        lhsT=S[:, kk * P : (kk + 1) * P],
                        rhs=G[:, kk * TF : kk * TF + F],
                        start=(kk == 0),
                        stop=(kk == CT - 1),
                    )
                epilogue(b, ps)
            # scale column APs for block b: scl[:, 4b+j : 4b+j+1]

        def rows_of(b):
            return min(P, n_loc - b * P)


        def sc(b, j):
            return scl[:, 4 * b + j : 4 * b + j + 1]

        # ---- layer 1, propagate #1: Tx1 = -Ds A Ds x ----
        def epi1(b, ps):
            rows = rows_of(b)
            tx1 = ev.tile([P, IN], f32, tag="tx1")
            nc.scalar.activation(tx1[:], ps[:], AF.Copy, scale=sc(b, 0))
            u1 = ev.tile([P, IN], bf16, tag="u1")
            nc.scalar.activation(u1[:], ps[:], AF.Copy, scale=sc(b, 1))
            nc.sync.dma_start(tx1_loc[:][b * P : b * P + P], tx1[:])
            nc.sync.dma_start(u1_loc[:][b * P : b * P + rows], u1[:rows])

        propagate(u0_d, IN, epi1)
        nc.gpsimd.collective_compute(
            "AllGather", OP.bypass, replica_groups=groups,
            ins=[u1_loc.opt()], outs=[u1_full.ap()],
        )

        # ---- layer 1, propagate #2 + dense layer 1 ----
        def epi2(b, ps):
            rows = rows_of(b)
            p2 = ev.tile([P, IN], f32, tag="p2")
            nc.scalar.activation(p2[:], ps[:], AF.Copy, scale=sc(b, 2))
            xT_t = io.tile([IN, P], f32, tag="xT_t")
            nc.sync.dma_start(xT_t[:], xT_d[:, b * P : (b + 1) * P])
            tx1 = io.tile([P, IN], f32, tag="tx1b")
            nc.sync.dma_start(tx1[:], tx1_loc[:][b * P : (b + 1) * P])
            outps = dps.tile([P, HID], f32, tag="dps")
            nc.tensor.matmul(
                out=outps[:], lhsT=xT_t[:], rhs=w1_t[0][:],
                start=True, stop=False, skip_group_check=True,
            )
            for k, t in [(1, tx1), (2, p2)]:
                tp = tps.tile([IN, P], f32, tag="trp")
                nc.tensor.transpose(tp[:], t[:], ident[:])
                tsb = ev.tile([IN, P], f32, tag="trs")
                nc.scalar.activation(tsb[:], tp[:], AF.Copy)
                nc.tensor.matmul(
                    out=outps[:], lhsT=tsb[:], rhs=w1_t[k][:],
                    start=False, stop=False, skip_group_check=True,
                )
            nc.tensor.matmul(
                out=outps[:], lhsT=ones1[:1, :], rhs=b1_t[:1, :],
                start=False, stop=True, skip_group_check=True,
            )
            h_t = ev.tile([P, HID], f32, tag="h_t")
            nc.scalar.activation(h_t[:], outps[:], AF.Relu)
            uh = ev.tile([P, TF], bf16, tag="uh")
            nc.vector.memset(uh[:, HID:TF], 0.0)
            nc.scalar.activation(uh[:, 0:HID], h_t[:], AF.Copy, scale=sc(b, 3))
            nc.sync.dma_start(h_loc[:][b * P : b * P + P], h_t[:])
            nc.sync.dma_start(uh_loc[:][b * P : b * P + rows], uh[:rows])

        propagate(u1_full.ap(), IN, epi2)
        nc.gpsimd.collective_compute(
            "AllGather", OP.bypass, replica_groups=groups,
            ins=[uh_loc.opt()], outs=[uh_full.ap()],
        )

        # ---- layer 2, propagate #1: Th1 ----
        def epi3(b, ps):
            rows = rows_of(b)
            th1 = ev.tile([P, HID], f32, tag="th1")
            nc.scalar.activation(th1[:], ps[:], AF.Copy, scale=sc(b, 0))
            ut1 = ev.tile([P, TF], bf16, tag="ut1")
            nc.vector.memset(ut1[:, HID:TF], 0.0)
            nc.scalar.activation(ut1[:, 0:HID], ps[:], AF.Copy, scale=sc(b, 1))
            nc.sync.dma_start(th1_loc[:][b * P : b * P + P], th1[:])
            nc.sync.dma_start(ut1_loc[:][b * P : b * P + rows], ut1[:rows])

        propagate(uh_full.ap(), HID, epi3)
        nc.gpsimd.collective_compute(
            "AllGather", OP.bypass, replica_groups=groups,
            ins=[ut1_loc.opt()], outs=[ut1_full.ap()],
        )

        # ---- layer 2, propagate #2 + dense layer 2 + output ----
        def epi4(b, ps):
            rows = rows_of(b)
            p4 = ev.tile([P, HID], f32, tag="p4")
            nc.scalar.activation(p4[:], ps[:], AF.Copy, scale=sc(b, 2))
            h_t = io.tile([P, HID], f32, tag="h_t2")
            nc.sync.dma_start(h_t[:], h_loc[:][b * P : (b + 1) * P])
            th1 = io.tile([P, HID], f32, tag="th1b")
            nc.sync.dma_start(th1[:], th1_loc[:][b * P : (b + 1) * P])
            outps = dps.tile([P, OUT], f32, tag="dps")
            for k, t in [(0, h_t), (1, th1), (2, p4)]:
                tp = tps.tile([HID, P], f32, tag="trp")
                nc.tensor.transpose(tp[:], t[:], ident[:])
                tsb = ev.tile([HID, P], f32, tag="trs2")
                nc.scalar.activation(tsb[:], tp[:], AF.Copy)
                nc.tensor.matmul(
                    out=outps[:], lhsT=tsb[:], rhs=w2_t[k][:],
                    start=(k == 0), stop=False, skip_group_check=True,
                )
            nc.tensor.matmul(
                out=outps[:], lhsT=ones1[:1, :], rhs=b2_t[:1, :],
                start=False, stop=True, skip_group_check=True,
            )
            o_t = ev.tile([P, OUT], f32, tag="o_t")
            nc.scalar.activation(o_t[:], outps[:], AF.Copy)
            nc.sync.dma_start(out_d[b * P : b * P + rows], o_t[:rows])

        propagate(ut1_full.ap(), HID, epi4)

    nc.compile()

    in_map = lambda c: {
        "u0": u0,
        "xT": np.ascontiguousarray(xT[c]),
        "gidx": np.ascontiguousarray(gidx[c]),
        "slots": np.ascontiguousarray(slots_bf[c]),
        "scales": np.ascontiguousarray(scales[c]),
        "w1": W1f,
        "b1": b1,
        "w2": W2f,
        "b2": b2,
    }
    in_maps = [in_map(c) for c in range(ncores)]
    return nc, in_maps


def build_and_run(cfg, x, edge_index, W1, b1, W2, b2, trace=False):
    from concourse.bass_utils import run_bass_kernel_spmd

    ncores = cfg["ncores"]
    nc, in_maps = build_program(cfg, x, edge_index, W1, b1, W2, b2)
    res = run_bass_kernel_spmd(nc, in_maps, list(range(ncores)), trace=trace)
    out = np.concatenate([res.results[c]["out"] for c in range(ncores)], axis=0)
    return out, res


def kernel(x, edge_index, W1, b1, W2, b2):
    cfg = dict(N=100000, E=1600000, IN=128, HID=64, OUT=40, ncores=8, gbufs=12, sbufs=6)
    trace = os.environ.get("CHEB_TRACE", "0") == "1"
    out, res = build_and_run(cfg, x, edge_index, W1, b1, W2, b2, trace=trace)
    if trace and res.exec_time_ns is not None:
        print(f"HW exec time: {res.exec_time_ns} ns")
    return out

